# revision 31
# baseline (speedup 1.0000x reference)
"""AlignmentContrastiveLoss on 8 Trainium2 NeuronCores.

Math notes (derived from the reference):
  - participating nodes are exactly those with >=1 positive partner, and every
    participating node is conserved. Within participating x participating,
    valid = (pos|neg)&part&~diag reduces to just ~same_graph.
  - the device computes U_i = sum_j exp(10*(E_i.E_j - PEN*[g_i==g_j])) over
    the gathered participating set; the -10*PEN logit penalty implements the
    mask and kills the diagonal. Everything else (positive-pair term, counts,
    log, final scalar) is O(N + pairs) host work.

v2 design (per core, SPMD-uniform; data decides the rest):
  - participants sorted by graph id -> the same-graph penalty region of any
    128-row m-tile fits inside its first ("diag") pair's 1024-col window, so
    ONE K=16 penalty matmul per diag pair replaces per-slot penalty matmuls.
  - triangle scheme: 112 slots (mi, ni) with ni >= mi//4 at 128x512
    granularity; slots paired per m-tile into 8 psum pairs per core.
    Positions 0..3 hold diag pairs (penalty MM, colsum on h1 only);
    positions 4..7 hold strictly-upper pairs (colsum h0+h1).
  - exp split across engines: ACT pairs use the real exp activation with
    fused row-sum accumulate; DVE pairs use a Schraudolph-style exp
    (affine to int16, bitcast to bf16) plus a 2-byte accumulate pass.
  - 12 colsum ones-matmuls land in one [128, 3*512] PSUM region (32-partition
    lanes) and leave via a single DMA; row-sums leave via one acc DMA.
"""

from contextlib import ExitStack

import ml_dtypes
import numpy as np

import bass_rust
import concourse.bass as bass
import concourse.mybir as mybir
import concourse.tile as tile
from concourse import bacc
from concourse.alu_op_type import AluOpType
from concourse.bass_utils import run_bass_kernel_spmd

N_CORES = 8
TEMP = 0.1
EPS = 1e-12
PEN = 2.0  # graph penalty; exp scale 1/T makes it -20 in logit space
NTILE = 512

# Schraudolph exp in bf16-bit space: i16 = round(A*x + B); bits(i16) as bf16
# approximate exp(10*x). A = 10*128*log2(e); B centers the multiplicative
# bias of the linear-mantissa interpolation (~ +4.6%) to ~zero mean.
_LOG2E = 1.4426950408889634
SCHRAUD_A = 10.0 * 128.0 * _LOG2E
SCHRAUD_C = 8.27  # bias-centering, in 1/128 exponent units
SCHRAUD_B = 128.0 * 127.0 - SCHRAUD_C

# engine per pair slot: True -> ACT (real exp), False -> DVE (Schraudolph)
ENG_ACT = (True, False, True, False, True, False, True, True)
# slots carrying a penalty matmul (diag pairs); slots are emitted in order,
# so colsum-heavy pairs come first and a light penalty pair ends the body
PEN_SLOTS = (2, 4, 6, 7)
N_PEN = len(PEN_SLOTS)


def _lane_seq():
    """PE colsum lanes in emission order; lane l lands at uc[l % 4, l // 4].
    The final slot's lane (7, 1) is handled by the Pool engine instead
    (partition_all_reduce straight from SBUF) to shorten the tail chain;
    its result lands at uc[:, 3, :]."""
    seq = [
        (p, h)
        for p in range(8)
        for h in (0, 1)
        if h == 1 or p not in PEN_SLOTS
    ]
    assert seq[-1] == (7, 1)
    return seq[:-1]

_programs: dict[tuple, bass.Bass] = {}


def _schraud_np(x):
    """Host-exact emulation of the DVE Schraudolph path (fp32 affine,
    round-to-nearest to int16, bits viewed as bf16)."""
    i = np.rint(np.float32(x) * np.float32(SCHRAUD_A) + np.float32(SCHRAUD_B))
    i = np.clip(i, -32768, 32767).astype(np.int16)
    return i.view(ml_dtypes.bfloat16).astype(np.float64)


PADVAL_DVE = float(_schraud_np(np.zeros(1))[0])  # exp-approx of logit 0
PADVAL_ACT = 1.0


def _tri_assign(npad: int):
    """Build the 8-core assignment. Returns per-core list of 8 entries
    (mi, [slot_half0, slot_half1]) where a slot half is an ni or None.

    PEN_SLOTS hold diag pairs (first pair of an m-tile; h0 = ni_d, h1 =
    ni_d+1 or None) or, for cores lacking a 4th diag pair, a single-slot
    strictly-upper pair packed with its slot at h1 (so the uniform
    "colsum on h1" covers it). Remaining slots: strictly-upper pairs.
    """
    m_t = npad // 128
    n_t = npad // NTILE
    diag, singles, others = [], [], []
    for mi in range(m_t):
        nis = list(range(mi // 4, n_t))
        first = nis[:2]
        diag.append((mi, [first[0], first[1] if len(first) > 1 else None]))
        rest = nis[2:]
        for a in range(0, len(rest), 2):
            grp = rest[a : a + 2]
            if len(grp) == 2:
                others.append((mi, [grp[0], grp[1]]))
            else:
                singles.append((mi, [None, grp[0]]))  # slot at h1
    assert len(diag) == m_t
    n_fill = 4 * N_CORES - len(diag)
    assert 0 <= n_fill <= len(singles), (len(diag), len(singles))
    pen_pool = diag + singles[:n_fill]
    rest_pool = singles[n_fill:]
    # rest pool singles: slot at h0 is fine too; keep h1 for uniform skipping
    rest_pool = rest_pool + others
    assert len(rest_pool) == 4 * N_CORES, len(rest_pool)
    cores = []
    for c in range(N_CORES):
        pens = [pen_pool[k * N_CORES + c] for k in range(4)]
        rests = [rest_pool[k * N_CORES + c] for k in range(4)]
        cores.append(
            [pens.pop(0) if p in PEN_SLOTS else rests.pop(0) for p in range(8)]
        )
    return cores


def _build_program_tri(npad: int, repeat: int = 1) -> bass.Bass:
    """SPMD program: 8 psum-pairs per core. Inputs per core:
      xs8  [128, 2, 8*128]  fp8 DoubleRow lhsT slabs (one m-tile per pair)
      ys8  [128, 2, 8*1024] fp8 rhs slabs (2 slot-halves per pair)
      xpen [16, 4*128]  bf16 penalty lhsT (-PEN * onehot of row graphs)
      ypen [16, 4*1024] bf16 penalty rhs (onehot of col graphs)
    Outputs:
      ur [128, 8] f32 row-sums per pair
      uc [4, 3, 512] f32 colsum lanes (partition-strided from PSUM)
    """
    bf = mybir.dt.bfloat16
    f8 = mybir.dt.float8e4
    f32 = mybir.dt.float32
    i16 = mybir.dt.int16
    Exp = mybir.ActivationFunctionType.Exp
    PPC = 8

    nc = bacc.Bacc(
        "TRN2",
        target_bir_lowering=False,
        debug=False,
        num_devices=N_CORES,
        disable_frame_to_traceback=True,
    )
    xs8p = nc.declare_dram_parameter("xs8p", [128, 2, PPC * 128], f8, isOutput=False)
    ys8p = nc.declare_dram_parameter("ys8p", [128, 2, PPC * 1024], f8, isOutput=False)
    xpenp = nc.declare_dram_parameter("xpenp", [16, N_PEN * 128], bf, isOutput=False)
    ypenp = nc.declare_dram_parameter("ypenp", [16, N_PEN * 1024], bf, isOutput=False)
    ur = nc.declare_dram_parameter("ur", [128, PPC + 1], f32, isOutput=True)
    uc = nc.declare_dram_parameter("uc", [4, 4 * NTILE], f32, isOutput=True)

    lane_seq = _lane_seq()
    assert len(lane_seq) == 11

    with tile.TileContext(nc) as tc, ExitStack() as ctx:
        const = ctx.enter_context(tc.tile_pool(name="const", bufs=1))
        psum = ctx.enter_context(
            tc.tile_pool(name="psum", bufs=2, space=bass.MemorySpace.PSUM)
        )
        psumc = ctx.enter_context(
            tc.tile_pool(name="psumc", bufs=1, space=bass.MemorySpace.PSUM)
        )
        scratch = ctx.enter_context(tc.tile_pool(name="scratch", bufs=2))
        accp = ctx.enter_context(tc.tile_pool(name="acc", bufs=2))

        # Warm the exp table while DMAs run.
        dummy_in = const.tile([128, 8], f32)
        nc.vector.memset(dummy_in[:], 0.0)
        dummy_out = const.tile([128, 8], bf)
        nc.scalar.activation(dummy_out[:], dummy_in[:], Exp)

        ones = const.tile([128, 32], bf)
        nc.vector.memset(ones[:], 1.0)

        xpen = const.tile([16, N_PEN * 128], bf)
        nc.sync.dma_start(xpen[:], xpenp[:, :])
        ypen = const.tile([16, N_PEN * 1024], bf)
        nc.sync.dma_start(ypen[:], ypenp[:, :])
        x8 = const.tile([128, 2, PPC * 128], f8)
        nc.sync.dma_start(x8[:], xs8p[:, :, :])
        # rhs slabs: finer at the head so compute starts early
        y8 = const.tile([128, 2, PPC * 1024], f8)
        bounds = [0, 1, 2, 4, 6, 8]
        for i in range(len(bounds) - 1):
            lo, hi = bounds[i] * 1024, bounds[i + 1] * 1024
            ring = nc.scalar if i % 2 == 0 else nc.sync
            ring.dma_start(y8[:, :, lo:hi], ys8p[:, :, lo:hi])

        def body():
            acc = accp.tile([128, PPC + 1], f32, tag="acc")
            colsb = scratch.tile([128, 4 * NTILE], f32, tag="colsb", bufs=1)
            dump = scratch.tile([128, 1024], bf, tag="dump", bufs=1)
            outs = {}
            lane_of = {ph: l for l, ph in enumerate(lane_seq)}
            cps_tiles = {}

            def emit_colsum(p, h):
                l = lane_of[(p, h)]
                g, sub = l // 4, l % 4
                if sub == 0:
                    cpst = psumc.tile([128, NTILE], f32, tag="cps", bufs=2)
                    cps_tiles[g] = cpst
                nc.tensor.matmul(
                    cps_tiles[g][32 * sub : 32 * (sub + 1), :],
                    ones[:, :32],
                    outs[(p, h)],
                    start=True, stop=True,
                    tile_position=(0, 32 * sub),
                )
                if l in (3, 7, 10):  # bank complete
                    gs = slice(g * NTILE, (g + 1) * NTILE)
                    nc.vector.tensor_copy(colsb[:, gs], cps_tiles[g][:])

            for p in range(PPC):
                ps = psum.tile([128, 1024], f32, tag="ps", bufs=3)
                for h in range(2):
                    nsl = slice(h * NTILE, (h + 1) * NTILE)
                    nc.tensor.matmul(
                        ps[:, nsl],
                        x8[:, :, p * 128 : (p + 1) * 128],
                        y8[:, :, (2 * p + h) * NTILE : (2 * p + h + 1) * NTILE],
                        start=True, stop=(p not in PEN_SLOTS),
                        perf_mode=mybir.MatmulPerfMode.DoubleRow,
                    )
                if p in PEN_SLOTS:
                    k = PEN_SLOTS.index(p)
                    for h in range(2):
                        nc.tensor.matmul(
                            ps[:, h * NTILE : (h + 1) * NTILE],
                            xpen[:, k * 128 : (k + 1) * 128],
                            ypen[:, k * 1024 + h * NTILE : k * 1024 + (h + 1) * NTILE],
                            start=False, stop=True,
                        )
                if p == PPC - 1:
                    # final slot: h1 first so the Pool colsum + uc DMA can
                    # overlap the h0 exp; row-sums split into acc cols p, p+1
                    sc = scratch.tile([128, 1024], bf, tag="sc", bufs=4)
                    nc.scalar.activation(
                        sc[:, NTILE:], ps[:, NTILE:], Exp,
                        scale=1.0 / TEMP,
                        accum_out=acc[:, p : p + 1],
                    )
                    nc.gpsimd.partition_all_reduce(
                        colsb[:, 3 * NTILE :], sc[:, NTILE:], 128,
                        bass_rust.ReduceOp.add,
                    )
                    nc.scalar.activation(
                        sc[:, :NTILE], ps[:, :NTILE], Exp,
                        scale=1.0 / TEMP,
                        accum_out=acc[:, p + 1 : p + 2],
                    )
                elif ENG_ACT[p]:
                    sc = scratch.tile([128, 1024], bf, tag="sc", bufs=4)
                    nc.scalar.activation(
                        sc[:], ps[:], Exp,
                        scale=1.0 / TEMP,
                        accum_out=acc[:, p : p + 1],
                    )
                    for h in range(2):
                        outs[(p, h)] = sc[:, h * NTILE : (h + 1) * NTILE]
                else:
                    t = scratch.tile([128, 1024], i16, tag="t", bufs=4)
                    nc.vector.tensor_scalar(
                        t[:], ps[:], SCHRAUD_A, SCHRAUD_B,
                        AluOpType.mult, AluOpType.add,
                    )
                    tb = t[:].bitcast(bf)
                    nc.vector.tensor_scalar(
                        dump[:], tb, 1.0, 0.0,
                        AluOpType.mult, AluOpType.add,
                        accum_out=acc[:, p : p + 1],
                    )
                    for h in range(2):
                        outs[(p, h)] = tb[:, h * NTILE : (h + 1) * NTILE]
                for h in range(2):
                    if (p, h) in lane_of:
                        emit_colsum(p, h)
            nc.sync.dma_start(uc[:, :], colsb[0:128:32, :])
            nc.sync.dma_start(ur[:, :], acc[:])

        if repeat == 1:
            body()
        else:
            with tc.For_i(0, repeat, 1):
                body()

    nc.compile()
    return nc


def _tri_in_maps(npad, yt8, gids):
    """Pack per-core operand slabs. yt8: [128, 2, npad] fp8 DoubleRow layout;
    gids: int graph id per padded column (gids[npp:] = -1)."""
    cores = _tri_assign(npad)
    f8np = yt8.dtype
    onehot = np.zeros((16, npad), dtype=ml_dtypes.bfloat16)
    real = gids >= 0
    onehot[gids[real], np.flatnonzero(real)] = 1.0
    in_maps = []
    for c in range(N_CORES):
        xs8p = np.zeros((128, 2, 8 * 128), dtype=f8np)
        ys8p = np.zeros((128, 2, 8 * 1024), dtype=f8np)
        xpenp = np.zeros((16, N_PEN * 128), dtype=ml_dtypes.bfloat16)
        ypenp = np.zeros((16, N_PEN * 1024), dtype=ml_dtypes.bfloat16)
        for p, (mi, halves) in enumerate(cores[c]):
            xs8p[:, :, p * 128 : (p + 1) * 128] = yt8[:, :, mi * 128 : (mi + 1) * 128]
            for h, ni in enumerate(halves):
                if ni is None:
                    continue
                s = 2 * p + h
                ys8p[:, :, s * NTILE : (s + 1) * NTILE] = yt8[
                    :, :, ni * NTILE : (ni + 1) * NTILE
                ]
            if p in PEN_SLOTS and halves[0] is not None and halves[0] == mi // 4:
                # true diag pair: penalty over its 1024-col window
                k = PEN_SLOTS.index(p)
                c0 = halves[0] * NTILE
                cw = min(1024, npad - c0)
                xpenp[:, k * 128 : (k + 1) * 128] = (
                    onehot[:, mi * 128 : (mi + 1) * 128].astype(np.float32) * -PEN
                ).astype(ml_dtypes.bfloat16)
                ypenp[:, k * 1024 : k * 1024 + cw] = onehot[:, c0 : c0 + cw]
        in_maps.append({"xs8p": xs8p, "ys8p": ys8p, "xpenp": xpenp, "ypenp": ypenp})
    return in_maps, cores


def _tri_combine(npad, npp, res, cores):
    """Scatter-add per-core row/col partial sums into U [npp], applying the
    deterministic pad/dummy corrections for exp(0)-valued placeholder cols."""
    lane_of = {ph: l for l, ph in enumerate(_lane_seq())}
    n_t = npad // NTILE
    u = np.zeros(npad, dtype=np.float64)
    for c in range(N_CORES):
        urr = res[c]["ur"].astype(np.float64)  # [128, 9]
        ucc = res[c]["uc"].astype(np.float64).reshape(4, 4, NTILE)
        for p, (mi, halves) in enumerate(cores[c]):
            padval = PADVAL_ACT if ENG_ACT[p] else PADVAL_DVE
            corr = 0.0
            for h, ni in enumerate(halves):
                if ni is None:
                    corr += NTILE * padval  # dummy half: S=0 everywhere
                elif ni == n_t - 1:
                    corr += (npad - npp) * padval  # pad cols in last n-tile
            rows = urr[:, p] - corr
            if p == len(ENG_ACT) - 1:
                rows = rows + urr[:, p + 1]  # final slot: split accumulators
            lo = mi * 128
            u[lo : lo + 128] += rows
            for h, ni in enumerate(halves):
                if ni is None or ni == mi // 4:
                    continue  # dummy or diag slot (mirror computed in-block)
                if (p, h) == (len(ENG_ACT) - 1, 1):
                    u[ni * NTILE : (ni + 1) * NTILE] += ucc[0, 3, :]
                    continue
                l = lane_of.get((p, h))
                if l is None:
                    continue
                u[ni * NTILE : (ni + 1) * NTILE] += ucc[l % 4, l // 4, :]
    return u[:npp]


def kernel(embeddings, labels, graph_ids, categories):
    emb = np.asarray(embeddings, dtype=np.float32)
    lab = np.asarray(labels).astype(np.int64)
    gid = np.asarray(graph_ids).astype(np.int64)
    cat = np.asarray(categories).astype(np.int64)
    n, d = emb.shape
    assert d == 256

    norms = np.linalg.norm(emb, axis=1, keepdims=True)
    e = emb / np.maximum(norms, EPS)

    cons = cat < 3

    # Label groups via sort; a conserved node participates iff its label group
    # has conserved members spanning >=2 distinct graphs.
    order = np.argsort(lab, kind="stable")
    lab_s = lab[order]
    starts = np.flatnonzero(np.r_[True, lab_s[1:] != lab_s[:-1]])
    ends = np.r_[starts[1:], n]

    part_mask = np.zeros(n, dtype=bool)
    cnt = np.zeros(n, dtype=np.int64)  # positive partners per node
    pair_i, pair_j = [], []
    for s, t in zip(starts, ends):
        idx = order[s:t]
        ci = idx[cons[idx]]
        if len(ci) < 2:
            continue
        gg = gid[ci]
        if (gg == gg[0]).all():
            continue
        part_mask[ci] = True
        gcounts = {}
        for g in gg:
            gcounts[g] = gcounts.get(g, 0) + 1
        cnt[ci] = len(ci) - np.array([gcounts[g] for g in gg])
        ii, jj = np.triu_indices(len(ci), k=1)
        diff = gg[ii] != gg[jj]
        pair_i.append(ci[ii[diff]])
        pair_j.append(ci[jj[diff]])

    if not pair_i:
        return np.float32(0.0)
    pair_i = np.concatenate(pair_i)
    pair_j = np.concatenate(pair_j)
    n_pairs = len(pair_i)
    if n_pairs == 0:
        return np.float32(0.0)

    s_pairs = np.einsum("ij,ij->i", e[pair_i], e[pair_j], dtype=np.float64)
    pos_loss = np.sum(1.0 - s_pairs) / n_pairs

    part = np.flatnonzero(part_mask)
    # sort participants by graph id so the same-graph penalty region of each
    # m-tile fits its diag pair's 1024-col window
    part = part[np.argsort(gid[part], kind="stable")]
    npp = len(part)
    npad = max(1024, -(-npp // NTILE) * NTILE)

    gids_pad = np.full(npad, -1, dtype=np.int64)
    gids_pad[:npp] = gid[part]

    f8np = mybir.dt.np(mybir.dt.float8e4)
    e8 = e[part].astype(f8np)
    yt8 = np.zeros((128, 2, npad), dtype=f8np)
    yt8[:, :, :npp] = e8.T.reshape(2, 128, npp).transpose(1, 0, 2)

    # coverage assertion for the 1024-col penalty window
    gcols = {}
    for j in range(npp):
        gcols.setdefault(gids_pad[j], [j, j])[1] = j
    for mi in range(npad // 128):
        lo, hi = mi * 128, min(mi * 128 + 128, npp)
        if lo >= npp:
            break
        for g in set(gids_pad[lo:hi]):
            assert gcols[g][1] < (mi // 4) * NTILE + 1024, (mi, g, gcols[g])

    in_maps, cores = _tri_in_maps(npad, yt8, gids_pad)
    key = (npad, "tri2")
    nc = _programs.get(key)
    if nc is None:
        nc = _build_program_tri(npad)
        _programs[key] = nc
    res = run_bass_kernel_spmd(nc, in_maps, core_ids=list(range(N_CORES)))
    u_full = _tri_combine(npad, npp, res.results, cores)

    lse = np.log(np.maximum(u_full, 1e-300))
    n_pos = 2 * n_pairs
    nce = (np.sum(cnt[part] * lse) - 2.0 * np.sum(s_pairs / TEMP)) / n_pos
    return np.float32(pos_loss + nce)
